# revision 76
# baseline (speedup 1.0000x reference)
"""Trainium2 Bass kernel for nn_EnhancedTransformerBlock_51917564674691.

Reference block (B=4, S=2048, D=256):
  x_global = global_mha(x, 8 heads, hd=32)          # dense S x S attention
  x_local  = local_mha(x, 4 heads, hd=64, window=5) # banded attention
  x_fused  = MLP_512(silu) over concat([x_global, x_local])
  x        = LN(x + x_fused); x = LN(x + FFN(x)); return x

Sharding: 8 cores = 4 batches x 2 sequence-halves. Each core computes the
full-batch K/V for global attention (needs all 2048 keys) and produces the
output for its 1024 tokens.

v3 design notes:
- The softmax exp (16.8M elems/core) can only run on ACT and DVE (GpSimd has
  no PSUM access). It is split between ACT's table exp (fp8e4 out) and an
  int8 Schraudolph affine on DVE (bitcast to the same fp8 bits); the ~3%
  staircase error cancels in softmax and is invisible at the output.
- fp8e4m3 + DoubleRow everywhere a K>=256 contraction allows: qkv/local
  projections (x and weights ship as fp8, weights x8 against subnormals),
  the fused-MLP and FFN gemms (weights x32/x16/x8, activations fp8 with
  power-of-2 prescales folded into Silu scale / LN epilogues). AV matmuls
  use fp8 operands but not DoubleRow (its dst must sit at partition 0,
  incompatible with the dual-group o/den packing).
- Attention out-projections are folded into fus_w1 host-side; value/out-proj
  biases ride through softmax into a folded b1'. g_oT/l_oT carry x32 so the
  tiny attention outputs clear the fp8 subnormal range.
- Two ACT-table phases: attention (Exp only) then MLP (one-op AF.Silu per
  gemm1 tile). LayerNorm istd uses a fast-inverse-sqrt bit trick + 2 Newton
  steps on DVE so no Ln/Exp table is needed in the MLP phase; the MLP runs
  as four interleaved 256-token chunks after attn, with SBUF-side
  elementwise spread onto the otherwise idle GpSimd.
- AV matmuls lag the exp front by two kt-pairs (and are demoted) so the
  in-order PE queue keeps delivering score tiles to the exp engines.
"""
import os
import numpy as np
import ml_dtypes

import concourse.bass as bass
import concourse.tile as tile
from concourse import bacc, mybir
from concourse.bass_utils import run_bass_kernel_spmd
from concourse.masks import make_identity

P = 128
BF = mybir.dt.bfloat16
F32 = mybir.dt.float32
FP8 = mybir.dt.float8e4
I8 = mybir.dt.int8
I32 = mybir.dt.int32
BF_NP = ml_dtypes.bfloat16
F8_NP = ml_dtypes.float8_e4m3fn

B, S, D = 4, 2048, 256
TQ = 1024           # tokens per core
XQ = 1152           # padded x_q length (own tokens + halo, zero padded)
NQT = 2             # global q tiles of 512
NKT = 16            # global key tiles of 128
GSC = 1.0 / np.sqrt(32.0)   # global attention scale
LSC = 0.125                 # local attention scale (1/sqrt(64))
LB = 124            # local block queries
NLB = 9             # local blocks (9*124 = 1116 >= 1024)
EPS = 1e-5

# Schraudolph exp -> fp8e4 bits: i8 = round(8*(GSC*s*log2e + 7 - c));
# bitcast to e4m3 gives exp(GSC*s) with ~±5% staircase error whose bias
# cancels in softmax (numerator and denominator share it).
LOG2E = 1.4426950408889634
WSC = 8.0            # projection-weight fp8 prescale; k/q/v psums carry x8
PSC = WSC * WSC      # score psum scale (both operands x8)
A8C = 8.0 * LOG2E * GSC / PSC
B8C = 8.0 * (7.0 - 0.02)
# engine per (key-tile, head-pair) exp instruction: ACT (table exp) or DVE
# (int8 Schraudolph affine). GpSimd cannot read PSUM on TRN2, so only these
# two engines can drain score psum. Chain p2=0 (scA) is all-ACT; chain p2=1
# (scB) mixes ACT/DVE so both engines stay busy across the WAR handoffs.
EXP_PAT = [["A", "A" if kt % 3 == 0 else "D"] for kt in range(16)]
GOS = 32.0          # g_oT / l_oT fp8 prescale (2^5, lifts attn outs out of
                    # the e4m3 subnormal range); folded into fus_w1 host-side
S1 = 1024.0         # fus gemm1 psum scale   (w1 x32, g/l_oT x32)
S2 = 8.0            # ffn gemm1 psum scale   (wn1 x8)
SW2 = 16.0          # fus/ffn gemm2 weight scale

AF = mybir.ActivationFunctionType
TT = mybir.AluOpType

# name -> (shape, np dtype) of per-core DRAM inputs (all SBUF-image [128, F])
# x and the qkv projection weights ship as fp8e4m3 (weights x8 to clear the
# subnormal range); the projection gemms run fp8 DoubleRow over both
# 128-row feature planes, and the x8 scales are folded into the exp /
# normalize constants downstream.
INPUT_SPECS = {
    "xkvT": ((P, 2 * 2048), F8_NP),   # x[b].T            (full batch, T-layout)
    "wgk": ((P, 2 * 256), F8_NP),
    "bgk": ((P, 2), np.float32),
    "wgq": ((P, 2 * 256), F8_NP),
    "bgq": ((P, 2), np.float32),
    "xqT": ((P, 2 * XQ), F8_NP),      # x_q.T padded      (own + halo, T-layout)
    "wgv": ((P, 2 * 256), F8_NP),
    "wtqk": ((P, 2 * 512), F8_NP),
    "btqk": ((P, 4), np.float32),
    "wtv": ((P, 2 * 256), F8_NP),
    "w1g": ((P, 2 * 512), F8_NP),     # fus_w1[:, :256] @ g_out_w * 32 (T-img)
    "w1t": ((P, 2 * 512), F8_NP),     # fus_w1[:, 256:] @ t_out_w * 32
    "wf2": ((P, 4 * 256), F8_NP),     # fus_w2 * 16
    "wn1": ((P, 2 * 512), F8_NP),     # ffn_w1 * gn_g * 8 (gain folded)
    "wn2": ((P, 4 * 256), F8_NP),     # ffn_w2 * 16
    "bf1": ((P, 4), np.float32),      # folded b1' (true scale, Silu bias)
    "bn1": ((P, 4), np.float32),      # folded bn1'
    "resN": ((P, 8 * 256), np.float32),   # x own tokens + fus_b2 (N-image)
    "g128": ((P, 256), np.float32),   # gn_g broadcast
    "b128": ((P, 256), np.float32),   # gn_b + ffn_b2 broadcast
    "fng128": ((P, 256), np.float32),
    "fnb128": ((P, 256), np.float32),
    "bandF": ((P, LB), BF_NP),        # band mask, first block (boundary baked)
    "bandM": ((P, LB), BF_NP),        # band mask, middle blocks
    "bandL": ((P, LB), BF_NP),        # band mask, last block
}


def _patch_act_tables():
    """Make Exp and Ln resolve to the combined natural_log_exp_and_others set
    so the table-load pass emits ONE load instead of thrashing between
    exp_and_others and natural_log."""
    import concourse.hw_specs as hs
    if getattr(hs, "_act_tables_patched", False):
        return
    orig = hs.get_activation_tables

    def patched(module_arch):
        t = dict(orig(module_arch))
        exp = mybir.ActivationFunctionType.Exp
        ln = mybir.ActivationFunctionType.Ln
        for name in ("exp_and_others", "exp_and_friends"):
            if name in t:
                t[name] = t[name] - {exp}
        if "natural_log" in t:
            t["natural_log"] = t["natural_log"] - {ln}
        return t

    hs.get_activation_tables = patched
    import concourse.bacc as bc
    bc.get_activation_tables = patched
    hs._act_tables_patched = True


def build():
    _patch_act_tables()
    nc = bacc.Bacc("TRN2", target_bir_lowering=False, debug=False, num_devices=8)
    dram = {}
    for name, (shape, npdt) in INPUT_SPECS.items():
        dram[name] = nc.dram_tensor(
            name, list(shape), mybir.dt.from_np(np.dtype(npdt)), kind="ExternalInput"
        ).ap()
    out_dram = nc.dram_tensor("out", [P, 8 * 256], F32, kind="ExternalOutput").ap()

    with tile.TileContext(nc) as tc:
        _emit(nc, tc, dram, out_dram)
    nc.compile()
    return nc


def _emit(nc, tc, dram, out_dram):
    from contextlib import ExitStack
    ctx = ExitStack()

    cpool = ctx.enter_context(tc.tile_pool(name="const", bufs=1))
    wpool = ctx.enter_context(tc.tile_pool(name="work", bufs=1))
    spool = ctx.enter_context(tc.tile_pool(name="scratch", bufs=4))
    epool = ctx.enter_context(tc.tile_pool(name="exps", bufs=2))
    pp = ctx.enter_context(tc.tile_pool(name="ps", bufs=1, space="PSUM"))

    def _kernel_body():
        # ---- load constants / inputs --------------------------------------
        cin = {}
        for name, (shape, npdt) in INPUT_SPECS.items():
            t = cpool.tile(list(shape), mybir.dt.from_np(np.dtype(npdt)), tag=name)
            nc.sync.dma_start(t[:], dram[name])
            cin[name] = t

        ones_bf = cpool.tile([P, 64], BF, tag="ones_bf")
        nc.gpsimd.memset(ones_bf[:], 1.0)
        c_one = cpool.tile([P, 1], I32, tag="c_one")
        nc.vector.memset(c_one[:], 1)
        c_magic = cpool.tile([P, 1], I32, tag="c_magic")
        nc.vector.memset(c_magic[:], 0x5f3759df)
        ident = cpool.tile([P, P], F32, tag="ident")
        make_identity(nc, ident[:])

        # reshaped views of inputs
        xkvT = cin["xkvT"][:].rearrange("p (k n) -> p k n", k=2)     # [128,2,2048]
        xqT = cin["xqT"][:].rearrange("p (k n) -> p k n", k=2)       # [128,2,1152]
        resN = cin["resN"][:].rearrange("p (t f) -> p t f", t=8)     # [128,8,256]
        w = {k: cin[k][:].rearrange("p (k2 n) -> p k2 n", k2=2)
             for k in ("wgq", "wgk", "wgv", "wtqk", "wtv", "w1g", "w1t", "wn1")}
        w["wf2"] = cin["wf2"][:].rearrange("p (k2 n) -> p k2 n", k2=4)
        w["wn2"] = cin["wn2"][:].rearrange("p (k2 n) -> p k2 n", k2=4)

        # ---- persistent intermediates ------------------------------------
        # qT_bd: block-diagonal queries [feat, hg, hc, q]: rows 32*hc..32*hc+32
        # hold head hc's features, other rows zero. Scores then contract the
        # FULL 128 rows of kT (no tile_position row-masking) so the PE's HAM
        # activity monitor sees full-array matmuls and keeps the clock at
        # 2.4 GHz (masked matmuls were measured to leave it throttled at 1.2).
        qT_bd = wpool.tile([P, 2, 4, 1024], BF, tag="qT_bd")
        nc.gpsimd.memset(qT_bd[:], 0.0)
        kT = wpool.tile([P, 2, 2048], BF, tag="kT")
        v_aug = wpool.tile([P, NKT, 8, 64], FP8, tag="v_aug")
        qkL = wpool.tile([P, 4, XQ], BF, tag="qkL")
        vL = wpool.tile([P, NLB, 256], BF, tag="vL")
        g_oT = wpool.tile([P, 2, 1024], FP8, tag="g_oT")
        l_oT = wpool.tile([P, 2, 1024], FP8, tag="l_oT")
        h1s = wpool.tile([P, 4, 1024], FP8, tag="h1s")
        x1N = wpool.tile([P, 8, 256], F32, tag="x1N")
        res2 = wpool.tile([P, 8, 256], F32, tag="res2")
        x1T = wpool.tile([P, 2, 1024], FP8, tag="x1T")
        h2s = wpool.tile([P, 4, 1024], FP8, tag="h2s")
        out_sb = wpool.tile([P, 8, 256], F32, tag="out_sb")

        # ones columns of v_aug (denominator trick); GpSimd, it's idle
        nc.gpsimd.memset(v_aug[:, :, :, 32:64], 1.0)

        # two independent half-score tiles so exp of one half overlaps
        # scores/AV of the other (pipelines the exp engines to ~full duty)
        def ps_scA():
            return pp.tile([P, 1024], F32, tag="scA", name="ps_scA")

        def ps_scB():
            return pp.tile([P, 1024], F32, tag="scB", name="ps_scB")

        def ps_av():
            return pp.tile([P, 512], F32, tag="av", bufs=2, name="ps_av")

        def ps_sm():
            return pp.tile([P, 512], F32, tag="sm", bufs=2, name="ps_sm")

        # ---- qkv projections (global) ------------------------------------
        # interleave kT/qT so the first global scores are ready ASAP
        def kT_tile(m, nt):
            pm = ps_sm()
            nc.tensor.matmul(pm[:], w["wgk"][:, :, 128 * m:128 * m + 128],
                             xkvT[:, :, 512 * nt:512 * nt + 512],
                             start=True, stop=True,
                             perf_mode=mybir.MatmulPerfMode.DoubleRow)
            if m == 0:
                # prologue: ACT is idle, use it; m1 happens mid-stream where
                # ACT is the bottleneck, so cast there on DVE
                nc.scalar.activation(kT[:, m, 512 * nt:512 * nt + 512], pm[:],
                                     AF.Identity, bias=cin["bgk"][:, m:m + 1])
            else:
                nc.vector.tensor_tensor(
                    kT[:, m, 512 * nt:512 * nt + 512], pm[:],
                    cin["bgk"][:, m:m + 1].to_broadcast([P, 512]), TT.add)

        def qT_tile(m, nt):
            pm = ps_sm()
            nc.tensor.matmul(pm[:], w["wgq"][:, :, 128 * m:128 * m + 128],
                             xqT[:, :, 2 + 512 * nt:2 + 512 * nt + 512],
                             start=True, stop=True,
                             perf_mode=mybir.MatmulPerfMode.DoubleRow)
            for hc in range(4):
                nc.vector.tensor_tensor(
                    qT_bd[32 * hc:32 * hc + 32, m, hc,
                          512 * nt:512 * nt + 512],
                    pm[32 * hc:32 * hc + 32, :],
                    cin["bgq"][32 * hc:32 * hc + 32, m:m + 1]
                    .to_broadcast([32, 512]),
                    TT.add)

        # m0 (heads 0-3) projections first: they gate the first global exp.
        # m1, local projections, and local attention are emitted later so
        # they fill engine gaps under the ACT-bound global-exp stream.
        kT_tile(0, 0)
        qT_tile(0, 0)
        kT_tile(0, 1)
        qT_tile(0, 1)
        kT_tile(0, 2)
        kT_tile(0, 3)

        def emit_vaug():
            # v (N-layout, ones-augmented): v[key, f] over full batch
            for mt in range(16):
                pm = ps_sm()
                nc.tensor.matmul(pm[:, 0:256], xkvT[:, :, 128 * mt:128 * mt + 128],
                                 w["wgv"][:, :, :], start=True, stop=True,
                                 perf_mode=mybir.MatmulPerfMode.DoubleRow)
                nc.vector.tensor_copy(
                    v_aug[:, mt, :, 0:32],
                    pm[:, 0:256].rearrange("p (h d) -> p h d", h=8))

        def emit_local_proj():
            for m in range(4):
                for nt in range(3):
                    pm = ps_sm()
                    nc.tensor.matmul(pm[:, 0:384],
                                     w["wtqk"][:, :, 128 * m:128 * m + 128],
                                     xqT[:, :, 384 * nt:384 * nt + 384],
                                     start=True, stop=True,
                                     perf_mode=mybir.MatmulPerfMode.DoubleRow)
                    nc.vector.tensor_tensor(
                        qkL[:, m, 384 * nt:384 * nt + 384], pm[:, 0:384],
                        cin["btqk"][:, m:m + 1].to_broadcast([P, 384]), TT.add)
            for blk in range(NLB):
                pm = ps_sm()
                nc.tensor.matmul(pm[:, 0:256], xqT[:, :, 124 * blk:124 * blk + 128],
                                 w["wtv"][:, :, :], start=True, stop=True,
                                 perf_mode=mybir.MatmulPerfMode.DoubleRow)
                nc.vector.tensor_copy(vL[:, blk, :], pm[:, 0:256])

        # ---- local attention ---------------------------------------------
        # Entirely on the sm tag (scores, AV, den) so it NEVER touches the
        # scA/scB/av slot queues that pace the global exp stream. Emitted
        # before attn(0): its small exps fill the otherwise-idle prologue
        # ACT, and its PE/DVE crumbs ride under the stream.
        def emit_local_block(blk):
            k0 = 124 * blk
            q0 = 2 + 124 * blk
            qn = 32 if blk == NLB - 1 else LB  # valid queries in this block
            band = cin["bandF"] if blk == 0 else (
                cin["bandL"] if blk == NLB - 1 else cin["bandM"])
            eloc = epool.tile([P, 4, LB], BF, tag="eloc", bufs=4)
            for l in range(4):
                r = l % 2
                pt = l // 2
                psl = ps_sm()
                nc.tensor.matmul(psl[:, 0:LB],
                                 qkL[64 * r:64 * r + 64, 2 + pt, k0:k0 + 128],
                                 qkL[64 * r:64 * r + 64, pt, q0:q0 + LB],
                                 start=True, stop=True, tile_position=(64 * r, 0))
                nc.scalar.activation(eloc[:, l, :], psl[:, 0:LB], AF.Exp,
                                     scale=LSC / PSC)
            nc.gpsimd.tensor_tensor(eloc[:], eloc[:],
                                    band[:, None, :].to_broadcast([P, 4, LB]),
                                    TT.mult)
            for pr in range(2):
                po = ps_sm()
                pd = ps_sm()
                for c in range(2):
                    l = 2 * pr + c
                    nc.tensor.matmul(po[64 * c:64 * c + 64, 0:LB],
                                     vL[:, blk, 64 * l:64 * l + 64], eloc[:, l, :],
                                     start=True, stop=True, tile_position=(0, 64 * c))
                    nc.tensor.matmul(pd[64 * c:64 * c + 64, 0:LB],
                                     ones_bf[:], eloc[:, l, :],
                                     start=True, stop=True, tile_position=(0, 64 * c))
                rec = spool.tile([P, LB], F32, tag="lrec", bufs=2)
                nc.vector.reciprocal_approx_fast(rec[:], pd[:, 0:LB])
                nc.vector.scalar_tensor_tensor(l_oT[:, pr, k0:k0 + qn],
                                               po[:, 0:qn], GOS / WSC,
                                               rec[:, 0:qn], TT.mult, TT.mult)

        # ---- global attention --------------------------------------------
        # One stream per (qt, hg, head-pair p2): 16 key tiles, one [128,1024]
        # exp per kt. Score psums alternate scA/scB by kt parity so the WAR
        # chain is two deep; the exp engine per kt follows EXP_PAT (ACT table
        # exp -> fp8, or int8-Schraudolph on DVE/GpSimd -> same fp8 bits).
        # AV contracts kt-pairs with fp8 DoubleRow matmuls (2 k-planes).
        def emit_attn(qt):
            qsl = slice(512 * qt, 512 * qt + 512)
            for hg in range(2):
                pav = [ps_av(), ps_av()]

                def emit_av(ktp, egs):
                    # pair p2 covers heads 4*hg+2p2, 4*hg+2p2+1:
                    #   psum rows 0:32 = o(head), 32:64 = den replicated,
                    #   rows 64:96 = o(head+1), 96:128 = den(head+1)
                    # (DoubleRow would halve this but requires dst partition
                    # 0 — the 64-offset dual-group packing is incompatible.)
                    # skip_group_check: CoreSim's zero-region tracker is
                    # partition-blind; the dual-group pattern is exact on HW.
                    for j in range(2):
                        kt = 2 * ktp + j
                        for p2 in range(2):
                            for c in range(2):
                                h = 4 * hg + 2 * p2 + c
                                nc.tensor.matmul(
                                    pav[p2][64 * c:64 * c + 64, :],
                                    v_aug[:, kt, h, :],
                                    egs[p2][:, j, 512 * c:512 * c + 512]
                                    .bitcast(FP8),
                                    start=(kt == 0), stop=(kt == NKT - 1),
                                    tile_position=(0, 64 * c),
                                    skip_group_check=True)

                # AV lags the exp front by two kt-pairs so the in-order PE
                # queue never waits on the freshest exps
                pend = []
                for ktp in range(NKT // 2):
                    egs = {}
                    for p2 in range(2):
                        egs[p2] = epool.tile(
                            [P, 2, 1024], I8, tag=f"eg{p2}",
                            name=f"eg{p2}", bufs=3)
                    for j in range(2):
                        kt = 2 * ktp + j
                        pscs = [ps_scA(), ps_scB()]
                        for p2 in range(2):
                            psc = pscs[p2]
                            for c in range(2):
                                hc = 2 * p2 + c
                                nc.tensor.matmul(
                                    psc[:, 512 * c:512 * c + 512],
                                    kT[:, hg, 128 * kt:128 * kt + 128],
                                    qT_bd[:, hg, hc, qsl],
                                    start=True, stop=True)
                            eng = EXP_PAT[kt][p2]
                            dst = egs[p2][:, j, :]
                            if eng == "A":
                                nc.scalar.activation(dst.bitcast(FP8),
                                                     psc[:], AF.Exp,
                                                     scale=GSC / PSC)
                            else:
                                nc.vector.tensor_scalar(
                                    dst, psc[:], A8C, B8C, TT.mult, TT.add)
                    pend.append((ktp, egs))
                    if len(pend) > 2:
                        # demote AV below the score/exp stream so ready
                        # scores keep feeding the exp engines
                        with tc.high_priority(offset=-5000):
                            emit_av(*pend.pop(0))
                for it in pend:
                    emit_av(*it)
                for p2 in range(2):
                    rec = spool.tile([P, 512], F32, tag="grec", bufs=2)
                    # recip of the whole bank; o-rows produce garbage that is
                    # never read (only den rows 32:64 / 96:128 are consumed)
                    nc.vector.reciprocal_approx_fast(rec[:], pav[p2][:])
                    nc.vector.scalar_tensor_tensor(
                        g_oT[64 * p2:64 * p2 + 32, hg, qsl],
                        pav[p2][0:32, :], GOS / WSC, rec[32:64, :], TT.mult, TT.mult)
                    nc.vector.scalar_tensor_tensor(
                        g_oT[64 * p2 + 32:64 * p2 + 64, hg, qsl],
                        pav[p2][64:96, :], GOS / WSC, rec[96:128, :], TT.mult, TT.mult)

        # ---- MLP tail per chunk ------------------------------------------
        # tail=True (last chunk): nothing left to hide ACT under, so shift
        # movable work (the +1, the Square+accum, transpose copies) from the
        # saturated DVE to the then-idle ScalarE.
        def silu(dst_ap, pm_ap, b_t, m, sc):
            """dst = silu(pm/sc + b) in one ACT op (Silu table); the psum
            arrives sc-scaled because the fp8 gemm1 weights carry
            power-of-2 prescales. b_t holds the true bias columns."""
            nc.scalar.activation(dst_ap, pm_ap, AF.Silu,
                                 bias=b_t[:, m:m + 1], scale=1.0 / sc)

        def g2_res_ln(tas, hsrc, w2, res_src, dest, xn_cb, tail):
            """gemm2 (contract 512 -> 256, N-layout out) + residual + LN core.

            dest[:, ta, :] = (xr - mu) * istd for ta in tas; xr = gemm2 + res.
            xn_cb(ta, xn_ap) post-processes the normalized tile.
            """
            xrs = []
            ntt = len(tas)
            mu_raw = spool.tile([P, 4], F32, tag="mu_raw")
            s2_raw = spool.tile([P, 4], F32, tag="s2_raw")
            for tt, ta in enumerate(tas):
                # tail: spread the token-tiles over av+sm so their LN stt
                # chains pipeline instead of serializing on 2 sm slots
                pm = ps_av() if (tail and tt % 2 == 0) else ps_sm()
                for k2 in range(2):
                    nc.tensor.matmul(pm[:, 0:256],
                                     hsrc[:, 2 * k2:2 * k2 + 2,
                                          128 * ta:128 * ta + 128],
                                     w2[:, 2 * k2:2 * k2 + 2, :],
                                     start=(k2 == 0), stop=(k2 == 1),
                                     perf_mode=mybir.MatmulPerfMode.DoubleRow)
                xr = spool.tile([P, 256], F32, tag=f"xr{tt}", name="xr", bufs=2)
                # psum is SW2-scaled (fp8 w2 carries x16)
                nc.vector.scalar_tensor_tensor(
                    xr[:], pm[:, 0:256], 1.0 / SW2, res_src(ta), TT.mult, TT.add,
                    accum_out=mu_raw[:, tt:tt + 1])
                sq = spool.tile([P, 256], F32, tag="sq", name="sq", bufs=2)
                nc.scalar.activation(sq[:], xr[:], AF.Square,
                                     accum_out=s2_raw[:, tt:tt + 1])
                xrs.append(xr)
            mu = spool.tile([P, 4], F32, tag="mu")
            mu2 = spool.tile([P, 4], F32, tag="mu2")
            var = spool.tile([P, 4], F32, tag="var")
            istd = spool.tile([P, 4], F32, tag="istd")
            nc.vector.tensor_scalar_mul(mu[:, 0:ntt], mu_raw[:, 0:ntt], 1.0 / 256.0)
            nc.vector.tensor_tensor(mu2[:, 0:ntt], mu[:, 0:ntt], mu[:, 0:ntt],
                                    TT.mult)
            nc.vector.scalar_tensor_tensor(var[:, 0:ntt], s2_raw[:, 0:ntt],
                                           1.0 / 256.0, mu2[:, 0:ntt],
                                           TT.mult, TT.subtract)
            nc.vector.tensor_scalar(var[:, 0:ntt], var[:, 0:ntt], 1.0, EPS,
                                    TT.mult, TT.add)
            # istd = rsqrt(var): fast-inverse-sqrt bit seed + 2 Newton steps
            # on DVE (avoids the Ln/Exp activation tables entirely, so the
            # MLP phase can own the Silu table with no thrash)
            t1i = spool.tile([P, 4], I32, tag="t1i")
            nc.vector.tensor_scalar(t1i[:, 0:ntt], var[:, 0:ntt].bitcast(I32),
                                    c_one[:, 0:1], None, TT.logical_shift_right)
            y0i = spool.tile([P, 4], I32, tag="y0i")
            nc.vector.tensor_tensor(y0i[:, 0:ntt],
                                    c_magic[:, 0:1].to_broadcast([P, ntt]),
                                    t1i[:, 0:ntt], TT.subtract)
            cur = y0i[:, 0:ntt].bitcast(F32)
            for it in range(2):
                aa = spool.tile([P, 4], F32, tag=f"nra{it}")
                nc.vector.tensor_tensor(aa[:, 0:ntt], cur, cur, TT.mult)
                zz = spool.tile([P, 4], F32, tag=f"nrz{it}")
                nc.vector.scalar_tensor_tensor(zz[:, 0:ntt], aa[:, 0:ntt], 1.0,
                                               var[:, 0:ntt], TT.mult, TT.mult)
                ww = spool.tile([P, 4], F32, tag=f"nrw{it}")
                nc.vector.tensor_scalar(ww[:, 0:ntt], zz[:, 0:ntt], -0.5, 1.5,
                                        TT.mult, TT.add)
                nxt = spool.tile([P, 4], F32, tag=f"nrn{it}")
                nc.vector.tensor_tensor(nxt[:, 0:ntt], cur, ww[:, 0:ntt],
                                        TT.mult)
                cur = nxt[:, 0:ntt]
            nc.vector.tensor_copy(istd[:, 0:ntt], cur)
            for tt, ta in enumerate(tas):
                if tail and tt % 2 == 0:
                    # Pool path: no scalar_tensor_tensor on GpSimd, so two
                    # tensor_tensor ops (it is idle in the tail)
                    nc.gpsimd.tensor_tensor(
                        dest[:, ta, :], xrs[tt][:],
                        mu[:, tt:tt + 1].to_broadcast([P, 256]), TT.subtract)
                    nc.gpsimd.tensor_tensor(
                        dest[:, ta, :], dest[:, ta, :],
                        istd[:, tt:tt + 1].to_broadcast([P, 256]), TT.mult)
                else:
                    nc.vector.scalar_tensor_tensor(
                        dest[:, ta, :], xrs[tt][:], mu[:, tt:tt + 1],
                        istd[:, tt:tt + 1].to_broadcast([P, 256]),
                        TT.subtract, TT.mult)
                xn_cb(ta, dest[:, ta, :])

        def mlp_chunk(q0, qw, tail):
            """MLP + FFN + both LNs for tokens [q0, q0+qw).

            The whole MLP runs as a tail phase after attention, so the
            attention psum tags (scA/scB/av) are free: gemm1 m-tiles spread
            over scA/scB halves and the token-tiles over av+sm.
            """
            qsl = slice(q0, q0 + qw)
            tas = list(range(q0 // 128, (q0 + qw) // 128))

            def g1_psums():
                pg = [ps_scA().rearrange("p (m n) -> p m n", m=2),
                      ps_scB().rearrange("p (m n) -> p m n", m=2)]
                return [pg[0][:, 0, 0:qw], pg[0][:, 1, 0:qw],
                        pg[1][:, 0, 0:qw], pg[1][:, 1, 0:qw]]

            # fused MLP gemm1 (out-projections folded in) + silu; fp8
            # DoubleRow contracts both 128-row feature planes per source
            g1p = g1_psums()
            for m in range(4):
                pm = g1p[m]
                nc.tensor.matmul(pm, w["w1g"][:, :, 128 * m:128 * m + 128],
                                 g_oT[:, :, qsl], start=True, stop=False,
                                 perf_mode=mybir.MatmulPerfMode.DoubleRow)
                nc.tensor.matmul(pm, w["w1t"][:, :, 128 * m:128 * m + 128],
                                 l_oT[:, :, qsl], start=False, stop=True,
                                 perf_mode=mybir.MatmulPerfMode.DoubleRow)
                silu(h1s[:, m, qsl], pm, cin["bf1"], m, S1)
            yield

            # gemm2 + residual + LN1 -> x1N (core), res2 = x1N*g + b
            def ln1_post(ta, xn_ap):
                veng = nc.gpsimd if (tail and ta % 2 == 0) else nc.vector
                veng.tensor_tensor(res2[:, ta, :], xn_ap, cin["g128"][:],
                                   TT.mult)
                veng.tensor_tensor(res2[:, ta, :], res2[:, ta, :],
                                   cin["b128"][:], TT.add)
                # transpose x1 chunk -> x1T for the FFN gemm (LN1 gain folded
                # into wn1 host-side, so transpose the core directly)
                for fh in range(2):
                    ptr = ps_sm()
                    nc.tensor.transpose(ptr[:, 0:128],
                                        x1N[:, ta, 128 * fh:128 * fh + 128],
                                        ident[:])
                    if fh == 0:
                        nc.scalar.activation(x1T[:, fh, 128 * ta:128 * ta + 128],
                                             ptr[:, 0:128], AF.Copy)
                    else:
                        nc.vector.tensor_copy(x1T[:, fh, 128 * ta:128 * ta + 128],
                                              ptr[:, 0:128])

            g2_res_ln(tas, h1s, w["wf2"], lambda ta: resN[:, ta, :], x1N,
                      ln1_post, tail)
            yield

            # FFN gemm1 + silu (fp8 DoubleRow over the 2 feature planes)
            g1p = g1_psums()
            for m in range(4):
                pm = g1p[m]
                nc.tensor.matmul(pm, w["wn1"][:, :, 128 * m:128 * m + 128],
                                 x1T[:, :, qsl], start=True, stop=True,
                                 perf_mode=mybir.MatmulPerfMode.DoubleRow)
                silu(h2s[:, m, qsl], pm, cin["bn1"], m, S2)
            yield

            # FFN gemm2 + residual(res2) + LN2 -> out_sb (with fn gain/bias)
            def ln2_post(ta, xn_ap):
                veng = nc.gpsimd if (tail and ta % 2 == 0) else nc.vector
                veng.tensor_tensor(xn_ap, xn_ap, cin["fng128"][:], TT.mult)
                veng.tensor_tensor(xn_ap, xn_ap, cin["fnb128"][:], TT.add)

            g2_res_ln(tas, h2s, w["wn2"], lambda ta: res2[:, ta, :], out_sb,
                      ln2_post, tail)

            nc.sync.dma_start(
                out_dram[:, 2 * q0:2 * q0 + 2 * qw],
                out_sb[:, tas[0]:tas[-1] + 1, :].rearrange("p t f -> p (t f)"))
            yield

        def mlp_all():
            # four quarter-chunks, phases interleaved in emission so the
            # psum-tag FIFOs rotate across chunks and the chains pipeline
            gens = [mlp_chunk(256 * i, 256, True) for i in range(4)]
            for ph in range(5):
                for g in gens:
                    next(g, None)

        # The pure-projection band (v_aug, local qkv) is demoted far below
        # the attention streams so its PE/DVE bursts never preempt the
        # stream's scores/AV — it fills genuine gaps only. m1 projections,
        # local attention and mlp(0) stay at normal (earlier-emitted = higher)
        # priority: their ACT work must preempt the exp stream occasionally
        # or they slide into the tail, which costs far more.
        with tc.high_priority(offset=-1000000):
            emit_vaug()
        # m1 projections (DVE casts) gate hg1: their sm-psum tiles must sit
        # EARLY in the sm slot FIFO (slot grants follow emission order), ahead
        # of the local projections whose demoted casts can lag under the
        # stream.
        for nt in range(4):
            kT_tile(1, nt)
        for nt in range(2):
            qT_tile(1, nt)
        emit_local_proj()
        for blk in range(NLB):
            emit_local_block(blk)
        emit_attn(0)
        emit_attn(1)
        mlp_all()

    REPEAT = int(os.environ.get("KREPEAT", "1"))
    if REPEAT > 1:
        with tc.For_i(0, REPEAT, 1):
            _kernel_body()
    else:
        _kernel_body()
    ctx.close()


# ======================================================================
# Host side
# ======================================================================

_NC = None


def _get_nc():
    global _NC
    if _NC is None:
        _NC = build()
    return _NC


def _img_T(mat):
    """[R, C] fp32 (R = k*128) -> SBUF image [128, k*C] for T-layout tiles."""
    R, C = mat.shape
    k = R // 128
    return np.ascontiguousarray(
        mat.reshape(k, 128, C).transpose(1, 0, 2).reshape(128, k * C))


def _img_N(mat):
    """[T, F] (T = t*128) -> SBUF image [128, t*F] for N-layout tiles."""
    T, F = mat.shape
    t = T // 128
    return np.ascontiguousarray(
        mat.reshape(t, 128, F).transpose(1, 0, 2).reshape(128, t * F))


def _bias_cols(b):
    """[k*128] -> [128, k] per-partition column layout."""
    return np.ascontiguousarray(b.reshape(-1, 128).T)


def _in_maps(x, g_in_w, g_in_b, g_out_w, g_out_b,
             t_in_w, t_in_b, t_out_w, t_out_b,
             fus_w1, fus_b1, fus_w2, fus_b2,
             ffn_w1, ffn_b1, ffn_w2, ffn_b2,
             gn_g, gn_b, fn_g, fn_b):
    x = np.asarray(x, np.float32)
    f32 = lambda a: np.asarray(a, np.float32)
    bf = lambda a: np.asarray(a, np.float32).astype(BF_NP)
    f8 = lambda a: np.asarray(a, np.float32).astype(F8_NP)

    g_in_w, g_in_b = f32(g_in_w), f32(g_in_b)
    t_in_w, t_in_b = f32(t_in_w), f32(t_in_b)
    g_out_w, g_out_b = f32(g_out_w), f32(g_out_b)
    t_out_w, t_out_b = f32(t_out_w), f32(t_out_b)
    fus_w1, fus_b1 = f32(fus_w1), f32(fus_b1)
    fus_w2, fus_b2 = f32(fus_w2), f32(fus_b2)
    ffn_w1, ffn_b1 = f32(ffn_w1), f32(ffn_b1)
    ffn_w2, ffn_b2 = f32(ffn_w2), f32(ffn_b2)
    gn_g, gn_b = f32(gn_g), f32(gn_b)
    fn_g, fn_b = f32(fn_g), f32(fn_b)

    # fold out-projections into fus_w1; value/out biases ride through softmax
    W1g = fus_w1[:, 0:256] @ g_out_w            # [512, 256]
    W1t = fus_w1[:, 256:512] @ t_out_w
    b1p = (fus_b1
           + fus_w1[:, 0:256] @ (g_out_w @ g_in_b[512:768] + g_out_b)
           + fus_w1[:, 256:512] @ (t_out_w @ t_in_b[512:768] + t_out_b))
    # fold LN1 gain/bias into FFN gemm1
    wn1p = ffn_w1 * gn_g[None, :]
    bn1p = ffn_b1 + ffn_w1 @ gn_b

    # shared (same on all cores) tensors
    shared = {
        "wgq": f8(_img_T(g_in_w[0:256].T) * WSC),
        "wgk": f8(_img_T(g_in_w[256:512].T) * WSC),
        "wgv": f8(_img_T(g_in_w[512:768].T) * WSC),
        "wtqk": f8(_img_T(t_in_w[0:512].T) * WSC),
        "wtv": f8(_img_T(t_in_w[512:768].T) * WSC),
        # fp8 weight images with power-of-2 prescales; the kernel divides
        # them back out inside the silu / LN epilogues
        "w1g": f8(_img_T(W1g.T) * (S1 / GOS)),
        "w1t": f8(_img_T(W1t.T) * (S1 / GOS)),
        "wf2": f8(_img_T(fus_w2.T) * SW2),
        "wn1": f8(_img_T(wn1p.T) * S2),
        "wn2": f8(_img_T(ffn_w2.T) * SW2),
        "bgq": _bias_cols(WSC * g_in_b[0:256]),
        "bgk": _bias_cols(WSC * g_in_b[256:512]),
        "btqk": _bias_cols(WSC * t_in_b[0:512]),
        "bf1": _bias_cols(b1p),
        "bn1": _bias_cols(bn1p),
        "g128": np.ascontiguousarray(np.broadcast_to(gn_g, (P, 256))),
        "b128": np.ascontiguousarray(np.broadcast_to(gn_b + ffn_b2, (P, 256))),
        "fng128": np.ascontiguousarray(np.broadcast_to(fn_g, (P, 256))),
        "fnb128": np.ascontiguousarray(np.broadcast_to(fn_b, (P, 256))),
    }
    # band mask: key row j valid for query qq iff qq <= j <= qq+4
    jj = np.arange(P)[:, None]
    qq = np.arange(LB)[None, :]
    bandA = ((qq <= jj) & (jj <= qq + 4)).astype(np.float32)

    in_maps = []
    for c in range(8):
        b, hh = c // 2, c % 2
        t0 = 1024 * hh
        xb = x[b]                                    # [2048, 256]
        xq = np.zeros((XQ + 4, D), np.float32)       # rows = x_q tokens t0-2 ..
        lo, hi = max(0, t0 - 2), min(S, t0 + XQ + 2)
        xq[lo - (t0 - 2):hi - (t0 - 2)] = xb[lo:hi]
        xq = xq[:XQ]                                 # guard: only XQ rows used
        bandF = bandA.copy()
        bandL = bandA.copy()
        if hh == 0:
            bandF[0:2] = 0.0        # keys at tokens -2, -1
        else:
            bandL[34:36] = 0.0      # block-8 keys x_q rows 1026, 1027 (= S, S+1)
        m = dict(shared)
        m["xkvT"] = f8(_img_T(xb.T))
        m["xqT"] = f8(_img_T(xq.T))
        m["resN"] = _img_N(xb[t0:t0 + 1024] + fus_b2[None, :])
        m["bandF"] = bandF.astype(BF_NP)
        m["bandM"] = bandA.astype(BF_NP)
        m["bandL"] = bandL.astype(BF_NP)
        in_maps.append(m)
    return in_maps


def _assemble(results):
    out = np.zeros((B, S, D), np.float32)
    for c in range(8):
        b, hh = c // 2, c % 2
        img = results[c]["out"]                      # [128, 2048]
        chunk = img.reshape(P, 8, 256).transpose(1, 0, 2).reshape(1024, 256)
        out[b, 1024 * hh:1024 * hh + 1024] = chunk
    return out


def kernel(**inputs):
    in_maps = _in_maps(**inputs)
    nc = _get_nc()
    res = run_bass_kernel_spmd(nc, in_maps, core_ids=list(range(8)))
    return _assemble(res.results)



# revision 80
# speedup vs baseline: 1.1276x; 1.1276x over previous
"""Trainium2 Bass kernel for nn_EnhancedTransformerBlock_51917564674691.

Reference block (B=4, S=2048, D=256):
  x_global = global_mha(x, 8 heads, hd=32)          # dense S x S attention
  x_local  = local_mha(x, 4 heads, hd=64, window=5) # banded attention
  x_fused  = MLP_512(silu) over concat([x_global, x_local])
  x        = LN(x + x_fused); x = LN(x + FFN(x)); return x

Sharding: 8 cores = 4 batches x 2 sequence-halves. Each core computes the
full-batch K/V for global attention (needs all 2048 keys) and produces the
output for its 1024 tokens.

v3 design notes:
- The softmax exp (16.8M elems/core) can only run on ACT and DVE (GpSimd has
  no PSUM access). It is split between ACT's table exp (fp8e4 out) and an
  int8 Schraudolph affine on DVE (bitcast to the same fp8 bits); the ~3%
  staircase error cancels in softmax and is invisible at the output.
- fp8e4m3 + DoubleRow everywhere a K>=256 contraction allows: qkv/local
  projections (x and weights ship as fp8, weights x8 against subnormals),
  the fused-MLP and FFN gemms (weights x32/x16/x8, activations fp8 with
  power-of-2 prescales folded into Silu scale / LN epilogues). AV matmuls
  use fp8 operands but not DoubleRow (its dst must sit at partition 0,
  incompatible with the dual-group o/den packing).
- Attention out-projections are folded into fus_w1 host-side; value/out-proj
  biases ride through softmax into a folded b1'. g_oT/l_oT carry x32 so the
  tiny attention outputs clear the fp8 subnormal range.
- Two ACT-table phases: attention (Exp only) then MLP (one-op AF.Silu per
  gemm1 tile). LayerNorm istd uses a fast-inverse-sqrt bit trick + 2 Newton
  steps on DVE so no Ln/Exp table is needed in the MLP phase; the MLP runs
  as four interleaved 256-token chunks after attn, with SBUF-side
  elementwise spread onto the otherwise idle GpSimd.
- AV matmuls lag the exp front by two kt-pairs (and are demoted) so the
  in-order PE queue keeps delivering score tiles to the exp engines.
"""
import os
import numpy as np
import ml_dtypes

import concourse.bass as bass
import concourse.tile as tile
from concourse import bacc, mybir
from concourse.bass_utils import run_bass_kernel_spmd
from concourse.masks import make_identity

P = 128
BF = mybir.dt.bfloat16
F32 = mybir.dt.float32
FP8 = mybir.dt.float8e4
I8 = mybir.dt.int8
I32 = mybir.dt.int32
BF_NP = ml_dtypes.bfloat16
F8_NP = ml_dtypes.float8_e4m3fn

B, S, D = 4, 2048, 256
TQ = 1024           # tokens per core
XQ = 1152           # padded x_q length (own tokens + halo, zero padded)
NQT = 2             # global q tiles of 512
NKT = 16            # global key tiles of 128
GSC = 1.0 / np.sqrt(32.0)   # global attention scale
LSC = 0.125                 # local attention scale (1/sqrt(64))
LB = 124            # local block queries
NLB = 9             # local blocks (9*124 = 1116 >= 1024)
EPS = 1e-5

# Schraudolph exp -> fp8e4 bits: i8 = round(8*(GSC*s*log2e + 7 - c));
# bitcast to e4m3 gives exp(GSC*s) with ~±5% staircase error whose bias
# cancels in softmax (numerator and denominator share it).
LOG2E = 1.4426950408889634
WSC = 8.0            # projection-weight fp8 prescale; k/q/v psums carry x8
PSC = WSC * WSC      # score psum scale (both operands x8)
A8C = 8.0 * LOG2E * GSC / PSC
B8C = 8.0 * (7.0 - 0.02)
# engine per (key-tile, head-pair) exp instruction: ACT (table exp) or DVE
# (int8 Schraudolph affine). GpSimd cannot read PSUM on TRN2, so only these
# two engines can drain score psum. Chain p2=0 (scA) is all-ACT; chain p2=1
# (scB) mixes ACT/DVE so both engines stay busy across the WAR handoffs.
EXP_PAT = [["A", "A" if kt % 3 == 0 else "D"] for kt in range(16)]
GOS = 32.0          # g_oT / l_oT fp8 prescale (2^5, lifts attn outs out of
                    # the e4m3 subnormal range); folded into fus_w1 host-side
S1 = 1024.0         # fus gemm1 psum scale   (w1 x32, g/l_oT x32)
S2 = 8.0            # ffn gemm1 psum scale   (wn1 x8)
SW2 = 16.0          # fus/ffn gemm2 weight scale

AF = mybir.ActivationFunctionType
TT = mybir.AluOpType

# name -> (shape, np dtype) of per-core DRAM inputs (all SBUF-image [128, F])
# x and the qkv projection weights ship as fp8e4m3 (weights x8 to clear the
# subnormal range); the projection gemms run fp8 DoubleRow over both
# 128-row feature planes, and the x8 scales are folded into the exp /
# normalize constants downstream.
INPUT_SPECS = {
    "xkvT": ((P, 2 * 2048), F8_NP),   # x[b].T            (full batch, T-layout)
    "wgk": ((P, 2 * 256), F8_NP),
    "bgk": ((P, 2), np.float32),
    "wgq": ((P, 2 * 256), F8_NP),
    "bgq": ((P, 2), np.float32),
    "xqT": ((P, 2 * XQ), F8_NP),      # x_q.T padded      (own + halo, T-layout)
    "wgv": ((P, 2 * 256), F8_NP),
    "wtqk": ((P, 2 * 512), F8_NP),
    "btqk": ((P, 4), np.float32),
    "wtv": ((P, 2 * 256), F8_NP),
    "w1g": ((P, 2 * 512), F8_NP),     # fus_w1[:, :256] @ g_out_w * 32 (T-img)
    "w1t": ((P, 2 * 512), F8_NP),     # fus_w1[:, 256:] @ t_out_w * 32
    "wf2": ((P, 4 * 256), F8_NP),     # fus_w2 * 16
    "wn1": ((P, 2 * 512), F8_NP),     # ffn_w1 * gn_g * 8 (gain folded)
    "wn2": ((P, 4 * 256), F8_NP),     # ffn_w2 * 16
    "bf1": ((P, 4), np.float32),      # folded b1' (true scale, Silu bias)
    "bn1": ((P, 4), np.float32),      # folded bn1'
    "resN": ((P, 8 * 256), np.float32),   # x own tokens + fus_b2 (N-image)
    "g128": ((P, 256), np.float32),   # gn_g broadcast
    "b128": ((P, 256), np.float32),   # gn_b + ffn_b2 broadcast
    "fng128": ((P, 256), np.float32),
    "fnb128": ((P, 256), np.float32),
    "bandF": ((P, LB), BF_NP),        # band mask, first block (boundary baked)
    "bandM": ((P, LB), BF_NP),        # band mask, middle blocks
    "bandL": ((P, LB), BF_NP),        # band mask, last block
}


def _patch_act_tables():
    """Make Exp and Ln resolve to the combined natural_log_exp_and_others set
    so the table-load pass emits ONE load instead of thrashing between
    exp_and_others and natural_log."""
    import concourse.hw_specs as hs
    if getattr(hs, "_act_tables_patched", False):
        return
    orig = hs.get_activation_tables

    def patched(module_arch):
        t = dict(orig(module_arch))
        exp = mybir.ActivationFunctionType.Exp
        ln = mybir.ActivationFunctionType.Ln
        for name in ("exp_and_others", "exp_and_friends"):
            if name in t:
                t[name] = t[name] - {exp}
        if "natural_log" in t:
            t["natural_log"] = t["natural_log"] - {ln}
        return t

    hs.get_activation_tables = patched
    import concourse.bacc as bc
    bc.get_activation_tables = patched
    hs._act_tables_patched = True


def build():
    _patch_act_tables()
    nc = bacc.Bacc("TRN2", target_bir_lowering=False, debug=False, num_devices=8)
    dram = {}
    for name, (shape, npdt) in INPUT_SPECS.items():
        dram[name] = nc.dram_tensor(
            name, list(shape), mybir.dt.from_np(np.dtype(npdt)), kind="ExternalInput"
        ).ap()
    out_dram = nc.dram_tensor("out", [P, 8 * 256], F32, kind="ExternalOutput").ap()

    with tile.TileContext(nc) as tc:
        _emit(nc, tc, dram, out_dram)
    nc.compile()
    return nc


def _emit(nc, tc, dram, out_dram):
    from contextlib import ExitStack
    ctx = ExitStack()

    cpool = ctx.enter_context(tc.tile_pool(name="const", bufs=1))
    wpool = ctx.enter_context(tc.tile_pool(name="work", bufs=1))
    spool = ctx.enter_context(tc.tile_pool(name="scratch", bufs=4))
    epool = ctx.enter_context(tc.tile_pool(name="exps", bufs=2))
    pp = ctx.enter_context(tc.tile_pool(name="ps", bufs=1, space="PSUM"))

    def _kernel_body():
        # ---- load constants / inputs --------------------------------------
        cin = {}
        for name, (shape, npdt) in INPUT_SPECS.items():
            t = cpool.tile(list(shape), mybir.dt.from_np(np.dtype(npdt)), tag=name)
            nc.sync.dma_start(t[:], dram[name])
            cin[name] = t

        ones_bf = cpool.tile([P, 64], BF, tag="ones_bf")
        nc.gpsimd.memset(ones_bf[:], 1.0)
        c_one = cpool.tile([P, 1], I32, tag="c_one")
        nc.vector.memset(c_one[:], 1)
        c_magic = cpool.tile([P, 1], I32, tag="c_magic")
        nc.vector.memset(c_magic[:], 0x5f3759df)
        ident = cpool.tile([P, P], F32, tag="ident")
        make_identity(nc, ident[:])

        # reshaped views of inputs
        xkvT = cin["xkvT"][:].rearrange("p (k n) -> p k n", k=2)     # [128,2,2048]
        xqT = cin["xqT"][:].rearrange("p (k n) -> p k n", k=2)       # [128,2,1152]
        resN = cin["resN"][:].rearrange("p (t f) -> p t f", t=8)     # [128,8,256]
        w = {k: cin[k][:].rearrange("p (k2 n) -> p k2 n", k2=2)
             for k in ("wgq", "wgk", "wgv", "wtqk", "wtv", "w1g", "w1t", "wn1")}
        w["wf2"] = cin["wf2"][:].rearrange("p (k2 n) -> p k2 n", k2=4)
        w["wn2"] = cin["wn2"][:].rearrange("p (k2 n) -> p k2 n", k2=4)

        # ---- persistent intermediates ------------------------------------
        # qT_bd: block-diagonal queries [feat, hg, hc, q]: rows 32*hc..32*hc+32
        # hold head hc's features, other rows zero. Scores then contract the
        # FULL 128 rows of kT (no tile_position row-masking) so the PE's HAM
        # activity monitor sees full-array matmuls and keeps the clock at
        # 2.4 GHz (masked matmuls were measured to leave it throttled at 1.2).
        qT_bd = wpool.tile([P, 2, 4, 1024], BF, tag="qT_bd")
        nc.gpsimd.memset(qT_bd[:], 0.0)
        kT = wpool.tile([P, 2, 2048], BF, tag="kT")
        v_aug = wpool.tile([P, NKT, 8, 64], FP8, tag="v_aug")
        qkL = wpool.tile([P, 4, XQ], BF, tag="qkL")
        vL = wpool.tile([P, NLB, 256], BF, tag="vL")
        g_oT = wpool.tile([P, 2, 1024], FP8, tag="g_oT")
        l_oT = wpool.tile([P, 2, 1024], FP8, tag="l_oT")
        h1s = wpool.tile([P, 4, 1024], FP8, tag="h1s")
        x1N = wpool.tile([P, 8, 256], F32, tag="x1N")
        res2 = wpool.tile([P, 8, 256], F32, tag="res2")
        x1T = wpool.tile([P, 2, 1024], FP8, tag="x1T")
        h2s = wpool.tile([P, 4, 1024], FP8, tag="h2s")
        out_sb = wpool.tile([P, 8, 256], F32, tag="out_sb")

        # ones columns of v_aug (denominator trick); GpSimd, it's idle
        nc.gpsimd.memset(v_aug[:, :, :, 32:64], 1.0)

        # two independent half-score tiles so exp of one half overlaps
        # scores/AV of the other (pipelines the exp engines to ~full duty)
        def ps_scA():
            return pp.tile([P, 1024], F32, tag="scA", name="ps_scA")

        def ps_scB():
            return pp.tile([P, 1024], F32, tag="scB", name="ps_scB")

        def ps_av():
            return pp.tile([P, 512], F32, tag="av", bufs=2, name="ps_av")

        def ps_sm():
            return pp.tile([P, 512], F32, tag="sm", bufs=2, name="ps_sm")

        # ---- qkv projections (global) ------------------------------------
        # interleave kT/qT so the first global scores are ready ASAP
        def kT_tile(m, nt):
            pm = ps_sm()
            nc.tensor.matmul(pm[:], w["wgk"][:, :, 128 * m:128 * m + 128],
                             xkvT[:, :, 512 * nt:512 * nt + 512],
                             start=True, stop=True,
                             perf_mode=mybir.MatmulPerfMode.DoubleRow)
            if m == 0:
                # prologue: ACT is idle, use it; m1 happens mid-stream where
                # ACT is the bottleneck, so cast there on DVE
                nc.scalar.activation(kT[:, m, 512 * nt:512 * nt + 512], pm[:],
                                     AF.Identity, bias=cin["bgk"][:, m:m + 1])
            else:
                nc.vector.tensor_tensor(
                    kT[:, m, 512 * nt:512 * nt + 512], pm[:],
                    cin["bgk"][:, m:m + 1].to_broadcast([P, 512]), TT.add)

        def qT_tile(m, nt):
            pm = ps_sm()
            nc.tensor.matmul(pm[:], w["wgq"][:, :, 128 * m:128 * m + 128],
                             xqT[:, :, 2 + 512 * nt:2 + 512 * nt + 512],
                             start=True, stop=True,
                             perf_mode=mybir.MatmulPerfMode.DoubleRow)
            for hc in range(4):
                nc.vector.tensor_tensor(
                    qT_bd[32 * hc:32 * hc + 32, m, hc,
                          512 * nt:512 * nt + 512],
                    pm[32 * hc:32 * hc + 32, :],
                    cin["bgq"][32 * hc:32 * hc + 32, m:m + 1]
                    .to_broadcast([32, 512]),
                    TT.add)

        # m0 (heads 0-3) projections first: they gate the first global exp.
        # m1, local projections, and local attention are emitted later so
        # they fill engine gaps under the ACT-bound global-exp stream.
        kT_tile(0, 0)
        qT_tile(0, 0)
        kT_tile(0, 1)
        qT_tile(0, 1)
        kT_tile(0, 2)
        kT_tile(0, 3)

        def emit_vaug():
            # v (N-layout, ones-augmented): v[key, f] over full batch
            for mt in range(16):
                pm = ps_sm()
                nc.tensor.matmul(pm[:, 0:256], xkvT[:, :, 128 * mt:128 * mt + 128],
                                 w["wgv"][:, :, :], start=True, stop=True,
                                 perf_mode=mybir.MatmulPerfMode.DoubleRow)
                nc.vector.tensor_copy(
                    v_aug[:, mt, :, 0:32],
                    pm[:, 0:256].rearrange("p (h d) -> p h d", h=8))

        def emit_local_proj():
            for m in range(4):
                for nt in range(3):
                    pm = ps_sm()
                    nc.tensor.matmul(pm[:, 0:384],
                                     w["wtqk"][:, :, 128 * m:128 * m + 128],
                                     xqT[:, :, 384 * nt:384 * nt + 384],
                                     start=True, stop=True,
                                     perf_mode=mybir.MatmulPerfMode.DoubleRow)
                    if nt % 2 == 0:
                        nc.scalar.activation(
                            qkL[:, m, 384 * nt:384 * nt + 384], pm[:, 0:384],
                            AF.Identity, bias=cin["btqk"][:, m:m + 1])
                    else:
                        nc.vector.tensor_tensor(
                            qkL[:, m, 384 * nt:384 * nt + 384], pm[:, 0:384],
                            cin["btqk"][:, m:m + 1].to_broadcast([P, 384]),
                            TT.add)
            for blk in range(NLB):
                pm = ps_sm()
                nc.tensor.matmul(pm[:, 0:256], xqT[:, :, 124 * blk:124 * blk + 128],
                                 w["wtv"][:, :, :], start=True, stop=True,
                                 perf_mode=mybir.MatmulPerfMode.DoubleRow)
                if blk % 2 == 0:
                    nc.scalar.activation(vL[:, blk, :], pm[:, 0:256], AF.Copy)
                else:
                    nc.vector.tensor_copy(vL[:, blk, :], pm[:, 0:256])

        # ---- local attention ---------------------------------------------
        # Entirely on the sm tag (scores, AV, den) so it NEVER touches the
        # scA/scB/av slot queues that pace the global exp stream. Emitted
        # before attn(0): its small exps fill the otherwise-idle prologue
        # ACT, and its PE/DVE crumbs ride under the stream.
        def emit_local_block(blk):
            k0 = 124 * blk
            q0 = 2 + 124 * blk
            qn = 32 if blk == NLB - 1 else LB  # valid queries in this block
            band = cin["bandF"] if blk == 0 else (
                cin["bandL"] if blk == NLB - 1 else cin["bandM"])
            eloc = epool.tile([P, 4, LB], BF, tag="eloc", bufs=4)
            for l in range(4):
                r = l % 2
                pt = l // 2
                psl = ps_sm()
                nc.tensor.matmul(psl[:, 0:LB],
                                 qkL[64 * r:64 * r + 64, 2 + pt, k0:k0 + 128],
                                 qkL[64 * r:64 * r + 64, pt, q0:q0 + LB],
                                 start=True, stop=True, tile_position=(64 * r, 0))
                nc.scalar.activation(eloc[:, l, :], psl[:, 0:LB], AF.Exp,
                                     scale=LSC / PSC)
            nc.gpsimd.tensor_tensor(eloc[:], eloc[:],
                                    band[:, None, :].to_broadcast([P, 4, LB]),
                                    TT.mult)
            for pr in range(2):
                po = ps_sm()
                pd = ps_sm()
                for c in range(2):
                    l = 2 * pr + c
                    nc.tensor.matmul(po[64 * c:64 * c + 64, 0:LB],
                                     vL[:, blk, 64 * l:64 * l + 64], eloc[:, l, :],
                                     start=True, stop=True, tile_position=(0, 64 * c))
                    nc.tensor.matmul(pd[64 * c:64 * c + 64, 0:LB],
                                     ones_bf[:], eloc[:, l, :],
                                     start=True, stop=True, tile_position=(0, 64 * c))
                rec = spool.tile([P, LB], F32, tag="lrec", bufs=2)
                nc.vector.reciprocal_approx_fast(rec[:], pd[:, 0:LB])
                nc.vector.scalar_tensor_tensor(l_oT[:, pr, k0:k0 + qn],
                                               po[:, 0:qn], GOS / WSC,
                                               rec[:, 0:qn], TT.mult, TT.mult)

        # ---- global attention --------------------------------------------
        # One stream per (qt, hg, head-pair p2): 16 key tiles, one [128,1024]
        # exp per kt. Score psums alternate scA/scB by kt parity so the WAR
        # chain is two deep; the exp engine per kt follows EXP_PAT (ACT table
        # exp -> fp8, or int8-Schraudolph on DVE/GpSimd -> same fp8 bits).
        # AV contracts kt-pairs with fp8 DoubleRow matmuls (2 k-planes).
        def emit_attn(qt):
            qsl = slice(512 * qt, 512 * qt + 512)
            for hg in range(2):
                pav = [ps_av(), ps_av()]

                def emit_av(ktp, egs):
                    # pair p2 covers heads 4*hg+2p2, 4*hg+2p2+1:
                    #   psum rows 0:32 = o(head), 32:64 = den replicated,
                    #   rows 64:96 = o(head+1), 96:128 = den(head+1)
                    # Head c=0 accumulates at partition 0, which is the one
                    # place fp8 DoubleRow is legal — it contracts the kt-pair
                    # in one instruction. Head c=1 (rows 64:128 via
                    # tile_position) must stay non-DR.
                    # skip_group_check: CoreSim's zero-region tracker is
                    # partition-blind; the dual-group pattern is exact on HW.
                    for p2 in range(2):
                        h0 = 4 * hg + 2 * p2
                        nc.tensor.matmul(
                            pav[p2][0:64, :],
                            v_aug[:, 2 * ktp:2 * ktp + 2, h0, :],
                            egs[p2][:, :, 0:512].bitcast(FP8),
                            start=(ktp == 0), stop=(ktp == NKT // 2 - 1),
                            perf_mode=mybir.MatmulPerfMode.DoubleRow,
                            skip_group_check=True)
                        for j in range(2):
                            kt = 2 * ktp + j
                            nc.tensor.matmul(
                                pav[p2][64:128, :],
                                v_aug[:, kt, h0 + 1, :],
                                egs[p2][:, j, 512:1024].bitcast(FP8),
                                start=(kt == 0), stop=(kt == NKT - 1),
                                tile_position=(0, 64),
                                skip_group_check=True)

                # AV lags the exp front by two kt-pairs so the in-order PE
                # queue never waits on the freshest exps
                pend = []
                for ktp in range(NKT // 2):
                    egs = {}
                    for p2 in range(2):
                        egs[p2] = epool.tile(
                            [P, 2, 1024], I8, tag=f"eg{p2}",
                            name=f"eg{p2}", bufs=3)
                    for j in range(2):
                        kt = 2 * ktp + j
                        pscs = [ps_scA(), ps_scB()]
                        for p2 in range(2):
                            psc = pscs[p2]
                            for c in range(2):
                                hc = 2 * p2 + c
                                nc.tensor.matmul(
                                    psc[:, 512 * c:512 * c + 512],
                                    kT[:, hg, 128 * kt:128 * kt + 128],
                                    qT_bd[:, hg, hc, qsl],
                                    start=True, stop=True)
                            eng = EXP_PAT[kt][p2]
                            dst = egs[p2][:, j, :]
                            if eng == "A":
                                nc.scalar.activation(dst.bitcast(FP8),
                                                     psc[:], AF.Exp,
                                                     scale=GSC / PSC)
                            else:
                                nc.vector.tensor_scalar(
                                    dst, psc[:], A8C, B8C, TT.mult, TT.add)
                    pend.append((ktp, egs))
                    if len(pend) > 2:
                        # demote AV below the score/exp stream so ready
                        # scores keep feeding the exp engines
                        with tc.high_priority(offset=-5000):
                            emit_av(*pend.pop(0))
                for it in pend:
                    emit_av(*it)
                for p2 in range(2):
                    rec = spool.tile([P, 512], F32, tag="grec", bufs=2)
                    # recip of the whole bank; o-rows produce garbage that is
                    # never read (only den rows 32:64 / 96:128 are consumed)
                    nc.vector.reciprocal_approx_fast(rec[:], pav[p2][:])
                    nc.vector.scalar_tensor_tensor(
                        g_oT[64 * p2:64 * p2 + 32, hg, qsl],
                        pav[p2][0:32, :], GOS / WSC, rec[32:64, :], TT.mult, TT.mult)
                    nc.vector.scalar_tensor_tensor(
                        g_oT[64 * p2 + 32:64 * p2 + 64, hg, qsl],
                        pav[p2][64:96, :], GOS / WSC, rec[96:128, :], TT.mult, TT.mult)

        # ---- MLP tail per chunk ------------------------------------------
        # tail=True (last chunk): nothing left to hide ACT under, so shift
        # movable work (the +1, the Square+accum, transpose copies) from the
        # saturated DVE to the then-idle ScalarE.
        def silu(dst_ap, pm_ap, b_t, m, sc):
            """dst = silu(pm/sc + b) in one ACT op (Silu table); the psum
            arrives sc-scaled because the fp8 gemm1 weights carry
            power-of-2 prescales. b_t holds the true bias columns."""
            nc.scalar.activation(dst_ap, pm_ap, AF.Silu,
                                 bias=b_t[:, m:m + 1], scale=1.0 / sc)

        def g2_res_ln(tas, hsrc, w2, res_src, dest, xn_cb, tail):
            """gemm2 (contract 512 -> 256, N-layout out) + residual + LN core.

            dest[:, ta, :] = (xr - mu) * istd for ta in tas; xr = gemm2 + res.
            xn_cb(ta, xn_ap) post-processes the normalized tile.
            """
            xrs = []
            ntt = len(tas)
            mu_raw = spool.tile([P, 4], F32, tag="mu_raw")
            s2_raw = spool.tile([P, 4], F32, tag="s2_raw")
            for tt, ta in enumerate(tas):
                # tail: spread the token-tiles over av+sm so their LN stt
                # chains pipeline instead of serializing on 2 sm slots
                pm = ps_av() if (tail and tt % 2 == 0) else ps_sm()
                for k2 in range(2):
                    nc.tensor.matmul(pm[:, 0:256],
                                     hsrc[:, 2 * k2:2 * k2 + 2,
                                          128 * ta:128 * ta + 128],
                                     w2[:, 2 * k2:2 * k2 + 2, :],
                                     start=(k2 == 0), stop=(k2 == 1),
                                     perf_mode=mybir.MatmulPerfMode.DoubleRow)
                xr = spool.tile([P, 256], F32, tag=f"xr{tt}", name="xr", bufs=2)
                # psum is SW2-scaled (fp8 w2 carries x16)
                nc.vector.scalar_tensor_tensor(
                    xr[:], pm[:, 0:256], 1.0 / SW2, res_src(ta), TT.mult, TT.add,
                    accum_out=mu_raw[:, tt:tt + 1])
                sq = spool.tile([P, 256], F32, tag="sq", name="sq", bufs=2)
                nc.scalar.activation(sq[:], xr[:], AF.Square,
                                     accum_out=s2_raw[:, tt:tt + 1])
                xrs.append(xr)
            mu = spool.tile([P, 4], F32, tag="mu")
            mu2 = spool.tile([P, 4], F32, tag="mu2")
            var = spool.tile([P, 4], F32, tag="var")
            istd = spool.tile([P, 4], F32, tag="istd")
            nc.vector.tensor_scalar_mul(mu[:, 0:ntt], mu_raw[:, 0:ntt], 1.0 / 256.0)
            nc.vector.tensor_tensor(mu2[:, 0:ntt], mu[:, 0:ntt], mu[:, 0:ntt],
                                    TT.mult)
            nc.vector.scalar_tensor_tensor(var[:, 0:ntt], s2_raw[:, 0:ntt],
                                           1.0 / 256.0, mu2[:, 0:ntt],
                                           TT.mult, TT.subtract)
            nc.vector.tensor_scalar(var[:, 0:ntt], var[:, 0:ntt], 1.0, EPS,
                                    TT.mult, TT.add)
            # istd = rsqrt(var): fast-inverse-sqrt bit seed + 2 Newton steps
            # on DVE (avoids the Ln/Exp activation tables entirely, so the
            # MLP phase can own the Silu table with no thrash)
            t1i = spool.tile([P, 4], I32, tag="t1i")
            nc.vector.tensor_scalar(t1i[:, 0:ntt], var[:, 0:ntt].bitcast(I32),
                                    c_one[:, 0:1], None, TT.logical_shift_right)
            y0i = spool.tile([P, 4], I32, tag="y0i")
            nc.vector.tensor_tensor(y0i[:, 0:ntt],
                                    c_magic[:, 0:1].to_broadcast([P, ntt]),
                                    t1i[:, 0:ntt], TT.subtract)
            cur = y0i[:, 0:ntt].bitcast(F32)
            for it in range(1):
                aa = spool.tile([P, 4], F32, tag=f"nra{it}")
                nc.vector.tensor_tensor(aa[:, 0:ntt], cur, cur, TT.mult)
                zz = spool.tile([P, 4], F32, tag=f"nrz{it}")
                nc.vector.scalar_tensor_tensor(zz[:, 0:ntt], aa[:, 0:ntt], 1.0,
                                               var[:, 0:ntt], TT.mult, TT.mult)
                ww = spool.tile([P, 4], F32, tag=f"nrw{it}")
                nc.vector.tensor_scalar(ww[:, 0:ntt], zz[:, 0:ntt], -0.5, 1.5,
                                        TT.mult, TT.add)
                nxt = spool.tile([P, 4], F32, tag=f"nrn{it}")
                nc.vector.tensor_tensor(nxt[:, 0:ntt], cur, ww[:, 0:ntt],
                                        TT.mult)
                cur = nxt[:, 0:ntt]
            nc.vector.tensor_copy(istd[:, 0:ntt], cur)
            for tt, ta in enumerate(tas):
                if tail and tt % 2 == 0:
                    # Pool path: no scalar_tensor_tensor on GpSimd, so two
                    # tensor_tensor ops (it is idle in the tail)
                    nc.gpsimd.tensor_tensor(
                        dest[:, ta, :], xrs[tt][:],
                        mu[:, tt:tt + 1].to_broadcast([P, 256]), TT.subtract)
                    nc.gpsimd.tensor_tensor(
                        dest[:, ta, :], dest[:, ta, :],
                        istd[:, tt:tt + 1].to_broadcast([P, 256]), TT.mult)
                else:
                    nc.vector.scalar_tensor_tensor(
                        dest[:, ta, :], xrs[tt][:], mu[:, tt:tt + 1],
                        istd[:, tt:tt + 1].to_broadcast([P, 256]),
                        TT.subtract, TT.mult)
                xn_cb(ta, dest[:, ta, :])

        def mlp_chunk(q0, qw, tail):
            """MLP + FFN + both LNs for tokens [q0, q0+qw).

            The whole MLP runs as a tail phase after attention, so the
            attention psum tags (scA/scB/av) are free: gemm1 m-tiles spread
            over scA/scB halves and the token-tiles over av+sm.
            """
            qsl = slice(q0, q0 + qw)
            tas = list(range(q0 // 128, (q0 + qw) // 128))

            def g1_psums():
                pg = [ps_scA().rearrange("p (m n) -> p m n", m=2),
                      ps_scB().rearrange("p (m n) -> p m n", m=2)]
                return [pg[0][:, 0, 0:qw], pg[0][:, 1, 0:qw],
                        pg[1][:, 0, 0:qw], pg[1][:, 1, 0:qw]]

            # fused MLP gemm1 (out-projections folded in) + silu; fp8
            # DoubleRow contracts both 128-row feature planes per source
            g1p = g1_psums()
            for m in range(4):
                pm = g1p[m]
                nc.tensor.matmul(pm, w["w1g"][:, :, 128 * m:128 * m + 128],
                                 g_oT[:, :, qsl], start=True, stop=False,
                                 perf_mode=mybir.MatmulPerfMode.DoubleRow)
                nc.tensor.matmul(pm, w["w1t"][:, :, 128 * m:128 * m + 128],
                                 l_oT[:, :, qsl], start=False, stop=True,
                                 perf_mode=mybir.MatmulPerfMode.DoubleRow)
                silu(h1s[:, m, qsl], pm, cin["bf1"], m, S1)
            yield

            # gemm2 + residual + LN1 -> x1N (core), res2 = x1N*g + b
            def ln1_post(ta, xn_ap):
                veng = nc.gpsimd if (tail and ta % 2 == 0) else nc.vector
                veng.tensor_tensor(res2[:, ta, :], xn_ap, cin["g128"][:],
                                   TT.mult)
                veng.tensor_tensor(res2[:, ta, :], res2[:, ta, :],
                                   cin["b128"][:], TT.add)
                # transpose x1 chunk -> x1T for the FFN gemm (LN1 gain folded
                # into wn1 host-side, so transpose the core directly)
                for fh in range(2):
                    ptr = ps_sm()
                    nc.tensor.transpose(ptr[:, 0:128],
                                        x1N[:, ta, 128 * fh:128 * fh + 128],
                                        ident[:])
                    if fh == 0:
                        nc.scalar.activation(x1T[:, fh, 128 * ta:128 * ta + 128],
                                             ptr[:, 0:128], AF.Copy)
                    else:
                        nc.vector.tensor_copy(x1T[:, fh, 128 * ta:128 * ta + 128],
                                              ptr[:, 0:128])

            g2_res_ln(tas, h1s, w["wf2"], lambda ta: resN[:, ta, :], x1N,
                      ln1_post, tail)
            yield

            # FFN gemm1 + silu (fp8 DoubleRow over the 2 feature planes)
            g1p = g1_psums()
            for m in range(4):
                pm = g1p[m]
                nc.tensor.matmul(pm, w["wn1"][:, :, 128 * m:128 * m + 128],
                                 x1T[:, :, qsl], start=True, stop=True,
                                 perf_mode=mybir.MatmulPerfMode.DoubleRow)
                silu(h2s[:, m, qsl], pm, cin["bn1"], m, S2)
            yield

            # FFN gemm2 + residual(res2) + LN2 -> out_sb (with fn gain/bias)
            def ln2_post(ta, xn_ap):
                veng = nc.gpsimd if (tail and ta % 2 == 0) else nc.vector
                veng.tensor_tensor(xn_ap, xn_ap, cin["fng128"][:], TT.mult)
                veng.tensor_tensor(xn_ap, xn_ap, cin["fnb128"][:], TT.add)

            g2_res_ln(tas, h2s, w["wn2"], lambda ta: res2[:, ta, :], out_sb,
                      ln2_post, tail)

            nc.sync.dma_start(
                out_dram[:, 2 * q0:2 * q0 + 2 * qw],
                out_sb[:, tas[0]:tas[-1] + 1, :].rearrange("p t f -> p (t f)"))
            yield

        def mlp_all():
            # four quarter-chunks, phases interleaved in emission so the
            # psum-tag FIFOs rotate across chunks and the chains pipeline
            gens = [mlp_chunk(256 * i, 256, True) for i in range(4)]
            for ph in range(5):
                for g in gens:
                    next(g, None)

        # The pure-projection band (v_aug, local qkv) is demoted far below
        # the attention streams so its PE/DVE bursts never preempt the
        # stream's scores/AV — it fills genuine gaps only. m1 projections,
        # local attention and mlp(0) stay at normal (earlier-emitted = higher)
        # priority: their ACT work must preempt the exp stream occasionally
        # or they slide into the tail, which costs far more.
        with tc.high_priority(offset=-1000000):
            emit_vaug()
        # m1 projections (DVE casts) gate hg1: their sm-psum tiles must sit
        # EARLY in the sm slot FIFO (slot grants follow emission order), ahead
        # of the local projections whose demoted casts can lag under the
        # stream.
        for nt in range(4):
            kT_tile(1, nt)
        for nt in range(2):
            qT_tile(1, nt)
        emit_local_proj()
        for blk in range(NLB):
            emit_local_block(blk)
        emit_attn(0)
        emit_attn(1)
        mlp_all()

    REPEAT = int(os.environ.get("KREPEAT", "1"))
    if REPEAT > 1:
        with tc.For_i(0, REPEAT, 1):
            _kernel_body()
    else:
        _kernel_body()
    ctx.close()


# ======================================================================
# Host side
# ======================================================================

_NC = None


def _get_nc():
    global _NC
    if _NC is None:
        _NC = build()
    return _NC


def _img_T(mat):
    """[R, C] fp32 (R = k*128) -> SBUF image [128, k*C] for T-layout tiles."""
    R, C = mat.shape
    k = R // 128
    return np.ascontiguousarray(
        mat.reshape(k, 128, C).transpose(1, 0, 2).reshape(128, k * C))


def _img_N(mat):
    """[T, F] (T = t*128) -> SBUF image [128, t*F] for N-layout tiles."""
    T, F = mat.shape
    t = T // 128
    return np.ascontiguousarray(
        mat.reshape(t, 128, F).transpose(1, 0, 2).reshape(128, t * F))


def _bias_cols(b):
    """[k*128] -> [128, k] per-partition column layout."""
    return np.ascontiguousarray(b.reshape(-1, 128).T)


def _in_maps(x, g_in_w, g_in_b, g_out_w, g_out_b,
             t_in_w, t_in_b, t_out_w, t_out_b,
             fus_w1, fus_b1, fus_w2, fus_b2,
             ffn_w1, ffn_b1, ffn_w2, ffn_b2,
             gn_g, gn_b, fn_g, fn_b):
    x = np.asarray(x, np.float32)
    f32 = lambda a: np.asarray(a, np.float32)
    bf = lambda a: np.asarray(a, np.float32).astype(BF_NP)
    f8 = lambda a: np.asarray(a, np.float32).astype(F8_NP)

    g_in_w, g_in_b = f32(g_in_w), f32(g_in_b)
    t_in_w, t_in_b = f32(t_in_w), f32(t_in_b)
    g_out_w, g_out_b = f32(g_out_w), f32(g_out_b)
    t_out_w, t_out_b = f32(t_out_w), f32(t_out_b)
    fus_w1, fus_b1 = f32(fus_w1), f32(fus_b1)
    fus_w2, fus_b2 = f32(fus_w2), f32(fus_b2)
    ffn_w1, ffn_b1 = f32(ffn_w1), f32(ffn_b1)
    ffn_w2, ffn_b2 = f32(ffn_w2), f32(ffn_b2)
    gn_g, gn_b = f32(gn_g), f32(gn_b)
    fn_g, fn_b = f32(fn_g), f32(fn_b)

    # fold out-projections into fus_w1; value/out biases ride through softmax
    W1g = fus_w1[:, 0:256] @ g_out_w            # [512, 256]
    W1t = fus_w1[:, 256:512] @ t_out_w
    b1p = (fus_b1
           + fus_w1[:, 0:256] @ (g_out_w @ g_in_b[512:768] + g_out_b)
           + fus_w1[:, 256:512] @ (t_out_w @ t_in_b[512:768] + t_out_b))
    # fold LN1 gain/bias into FFN gemm1
    wn1p = ffn_w1 * gn_g[None, :]
    bn1p = ffn_b1 + ffn_w1 @ gn_b

    # shared (same on all cores) tensors
    shared = {
        "wgq": f8(_img_T(g_in_w[0:256].T) * WSC),
        "wgk": f8(_img_T(g_in_w[256:512].T) * WSC),
        "wgv": f8(_img_T(g_in_w[512:768].T) * WSC),
        "wtqk": f8(_img_T(t_in_w[0:512].T) * WSC),
        "wtv": f8(_img_T(t_in_w[512:768].T) * WSC),
        # fp8 weight images with power-of-2 prescales; the kernel divides
        # them back out inside the silu / LN epilogues
        "w1g": f8(_img_T(W1g.T) * (S1 / GOS)),
        "w1t": f8(_img_T(W1t.T) * (S1 / GOS)),
        "wf2": f8(_img_T(fus_w2.T) * SW2),
        "wn1": f8(_img_T(wn1p.T) * S2),
        "wn2": f8(_img_T(ffn_w2.T) * SW2),
        "bgq": _bias_cols(WSC * g_in_b[0:256]),
        "bgk": _bias_cols(WSC * g_in_b[256:512]),
        "btqk": _bias_cols(WSC * t_in_b[0:512]),
        "bf1": _bias_cols(b1p),
        "bn1": _bias_cols(bn1p),
        "g128": np.ascontiguousarray(np.broadcast_to(gn_g, (P, 256))),
        "b128": np.ascontiguousarray(np.broadcast_to(gn_b + ffn_b2, (P, 256))),
        "fng128": np.ascontiguousarray(np.broadcast_to(fn_g, (P, 256))),
        "fnb128": np.ascontiguousarray(np.broadcast_to(fn_b, (P, 256))),
    }
    # band mask: key row j valid for query qq iff qq <= j <= qq+4
    jj = np.arange(P)[:, None]
    qq = np.arange(LB)[None, :]
    bandA = ((qq <= jj) & (jj <= qq + 4)).astype(np.float32)

    in_maps = []
    for c in range(8):
        b, hh = c // 2, c % 2
        t0 = 1024 * hh
        xb = x[b]                                    # [2048, 256]
        xq = np.zeros((XQ + 4, D), np.float32)       # rows = x_q tokens t0-2 ..
        lo, hi = max(0, t0 - 2), min(S, t0 + XQ + 2)
        xq[lo - (t0 - 2):hi - (t0 - 2)] = xb[lo:hi]
        xq = xq[:XQ]                                 # guard: only XQ rows used
        bandF = bandA.copy()
        bandL = bandA.copy()
        if hh == 0:
            bandF[0:2] = 0.0        # keys at tokens -2, -1
        else:
            bandL[34:36] = 0.0      # block-8 keys x_q rows 1026, 1027 (= S, S+1)
        m = dict(shared)
        m["xkvT"] = f8(_img_T(xb.T))
        m["xqT"] = f8(_img_T(xq.T))
        m["resN"] = _img_N(xb[t0:t0 + 1024] + fus_b2[None, :])
        m["bandF"] = bandF.astype(BF_NP)
        m["bandM"] = bandA.astype(BF_NP)
        m["bandL"] = bandL.astype(BF_NP)
        in_maps.append(m)
    return in_maps


def _assemble(results):
    out = np.zeros((B, S, D), np.float32)
    for c in range(8):
        b, hh = c // 2, c % 2
        img = results[c]["out"]                      # [128, 2048]
        chunk = img.reshape(P, 8, 256).transpose(1, 0, 2).reshape(1024, 256)
        out[b, 1024 * hh:1024 * hh + 1024] = chunk
    return out


def kernel(**inputs):
    in_maps = _in_maps(**inputs)
    nc = _get_nc()
    res = run_bass_kernel_spmd(nc, in_maps, core_ids=list(range(8)))
    return _assemble(res.results)

